# revision 1
# baseline (speedup 1.0000x reference)
"""Gaussian density-grid kernel for Trainium2 (8 NeuronCores).

density[g] = sum_{a,n} aw[a,n]*mask[a] * exp(bw[a,n] * ||grid_g - X_a||^2)

The grid is a regular 48^3 lattice, so the Gaussian factorizes per axis:
    exp(bw*(dx^2+dy^2+dz^2)) = Ex(i) * Ey(j) * Ez(k)
Per (atom, gaussian) pair p we build three 48-entry 1D tables, form the
outer product Ey (x) Ez on the vector engine, and contract over p with the
tensor engine:  out[i, (k,j)] = sum_p (w*Ex)[p,i] * (Ey*Ez)[p,(k,j)].

Active (mask==1) atoms are compacted on the host and the p axis is sharded
across the 8 cores; each core produces a partial density over the full grid
and the host sums the 8 partials.
"""

import math

import numpy as np

NXYZ = 48
G2D = NXYZ * NXYZ  # 2304 (k,j) pairs
G = NXYZ * G2D
N_CORES = 8
P_TILE = 128
NCOEF = 5  # s, -s*x, -s*y, -s*z, log(w) per chunk
ZBLK = 8  # z-rows per M piece -> N = ZBLK*48 = 384 per matmul
NPIECE = NXYZ // ZBLK  # 6 matmul pieces per chunk

# matmul operand dtype: "f32r" (1 cyc/row, reduced-precision multiply),
# "f32" (4 cyc/row, exact) or "bf16"
import os as _os

MM_DTYPE = _os.environ.get("DENS_MM_DTYPE", "f32r")


def _build_program(n_chunks: int):
    import concourse.mybir as mybir
    import concourse.tile as tile
    from concourse import bacc
    from concourse.alu_op_type import AluOpType
    from concourse.tile_rust import add_dep_helper

    f32 = mybir.dt.float32
    ACT = mybir.ActivationFunctionType

    nc = bacc.Bacc(
        "TRN2",
        target_bir_lowering=False,
        debug=False,
        enable_asserts=False,
        num_devices=N_CORES,
    )

    # packed input: [coef (NCOEF*n_chunks) | ay | az | ax]
    inp_w = 3 * NXYZ + NCOEF * n_chunks
    wa = NCOEF * n_chunks + 2 * NXYZ  # first DMA: coef + ay + az
    inp_d = nc.dram_tensor("inp", [P_TILE, inp_w], f32, kind="ExternalInput")
    dens_d = nc.dram_tensor("dens", [G], f32, kind="ExternalOutput")

    if MM_DTYPE == "bf16":
        mm_dt = mybir.dt.bfloat16
    elif MM_DTYPE == "f32r":
        mm_dt = mybir.dt.float32r
    else:
        mm_dt = f32

    with tile.TileContext(nc) as tc:
        with (
            tc.tile_pool(name="const", bufs=1) as cpool,
            tc.tile_pool(name="work", bufs=3) as wpool,
            tc.tile_pool(name="mbuf", bufs=2) as mpool,
            tc.tile_pool(name="outs", bufs=1) as opool,
            tc.tile_pool(name="acc", bufs=1, space="PSUM") as acc_pool,
        ):
            # dummy activation with no data deps: forces the ACT table load
            # to issue before the DMA-wait blocks the Scalar queue
            dummy = cpool.tile([P_TILE, 1], f32)
            nc.scalar.activation(dummy[:], dummy[:], ACT.Exp, bias=0.0, scale=0.0)

            # two tiles (not one) so the two DMAs have no false WAW dep
            inp_a = cpool.tile([P_TILE, wa], f32)
            nc.sync.dma_start(inp_a[:], inp_d.ap()[:, 0:wa])
            inp_b = cpool.tile([P_TILE, NXYZ], f32)
            nc.gpsimd.dma_start(inp_b[:], inp_d.ap()[:, wa:inp_w])
            ay_b = inp_a[:, NCOEF * n_chunks : NCOEF * n_chunks + NXYZ]
            az_b = inp_a[:, NCOEF * n_chunks + NXYZ : NCOEF * n_chunks + 2 * NXYZ]
            ax_b = inp_b[:, 0:NXYZ]
            coef_off = 0

            # PE warm-up: ~3.5us of dep-free matmuls on zeroed scratch while
            # waiting for the input DMA, so the HAM un-throttles the PE clock
            # (1.2 -> 2.4 GHz) before the real matmuls that gate the drains
            warm_l = cpool.tile([P_TILE, NXYZ], mm_dt, name="warm_l")
            warm_r = cpool.tile([P_TILE, ZBLK * NXYZ], mm_dt, name="warm_r")
            nc.vector.memset(warm_l[:].bitcast(f32), 0.0)
            nc.vector.memset(warm_r[:].bitcast(f32), 0.0)
            warm_ps = acc_pool.tile(
                [NXYZ, ZBLK * NXYZ], f32, tag="warmps", name="warmps"
            )
            for _ in range(12):
                nc.tensor.matmul(warm_ps[:], warm_l[:], warm_r[:], start=True, stop=True)

            # PSUM accumulators [48, 384] per z-piece, accumulated over chunks
            accs = [
                acc_pool.tile([NXYZ, ZBLK * NXYZ], f32, tag=f"acc{b}", name=f"acc{b}")
                for b in range(NPIECE)
            ]

            # Stage 1: per-chunk 1D Gaussian tables (kept alive for all pieces)
            exs, eyzs = [], []
            for c in range(n_chunks):
                o = coef_off + c * NCOEF
                s_c = inp_a[:, o : o + 1]
                ntx = inp_a[:, o + 1 : o + 2]
                nty = inp_a[:, o + 2 : o + 3]
                ntz = inp_a[:, o + 3 : o + 4]
                lw = inp_a[:, o + 4 : o + 5]

                # SQ[:, axis_block] = (s*coord - s*center)^2 = -bw * d^2
                # yz first so the M outer-product (needs eyz only) starts early
                sq = wpool.tile([P_TILE, 3 * NXYZ], f32, tag="sq", name=f"sq{c}")
                if c == 0:
                    # chunk 0 on the idle-at-start Vector engine (tensor_scalar
                    # runs 2x for fp32) to shorten the ACT prefix
                    u = wpool.tile([P_TILE, 2 * NXYZ], f32, tag="u", name="u0")
                    nc.vector.tensor_scalar(
                        u[:, 0:NXYZ], ay_b, s_c, nty, AluOpType.mult, AluOpType.add
                    )
                    nc.vector.tensor_scalar(
                        u[:, NXYZ : 2 * NXYZ], az_b, s_c, ntz, AluOpType.mult, AluOpType.add
                    )
                    nc.vector.tensor_tensor(
                        sq[:, NXYZ : 3 * NXYZ], u[:], u[:], AluOpType.mult
                    )
                else:
                    for blk, bias, coord in ((1, nty, ay_b), (2, ntz, az_b)):
                        sq_i = nc.scalar.activation(
                            sq[:, blk * NXYZ : (blk + 1) * NXYZ],
                            coord,
                            ACT.Square,
                            bias=bias,
                            scale=s_c,
                        )
                        # keep later-chunk ACT work behind chunk 0's critical
                        # Exp so it can't jump the in-order ACT queue
                        add_dep_helper(
                            sq_i.ins, exp_yz0.ins, sync=False, reason="act order"
                        )
                eyz_dt = mm_dt if MM_DTYPE == "bf16" else f32
                eyz = wpool.tile(
                    [P_TILE, 2 * NXYZ], eyz_dt, tag="eyz", name=f"eyz{c}", bufs=n_chunks
                )
                exp_yz = nc.scalar.activation(
                    eyz[:], sq[:, NXYZ : 3 * NXYZ], ACT.Exp, bias=0.0, scale=-1.0
                )
                if c == 0:
                    exp_yz0 = exp_yz
                # Ex = exp(-SQx + log w)  (weight folded in)
                sqx_i = nc.scalar.activation(
                    sq[:, 0:NXYZ], ax_b, ACT.Square, bias=ntx, scale=s_c
                )
                if c > 0:
                    add_dep_helper(sqx_i.ins, exp_yz0.ins, sync=False, reason="act order")
                ex = wpool.tile([P_TILE, NXYZ], mm_dt, tag="ex", name=f"ex{c}", bufs=n_chunks)
                nc.scalar.activation(ex[:], sq[:, 0:NXYZ], ACT.Exp, bias=lw, scale=-1.0)
                exs.append(ex)
                eyzs.append(eyz)

            # Stage 2: c-outer sweeps (no DVE head-of-line stall on chunk-1
            # tables); piece b drains during the final sweep right after its
            # stop matmul, overlapping the remaining pieces' work
            out_s = opool.tile([NXYZ, G2D], f32)
            for c in range(n_chunks):
                for b in range(NPIECE):
                    ey = eyzs[c][:, 0:NXYZ]
                    ez_b = eyzs[c][:, NXYZ + b * ZBLK : NXYZ + (b + 1) * ZBLK]
                    m_t = mpool.tile(
                        [P_TILE, ZBLK * NXYZ], mm_dt, tag="m", name=f"m{b}_{c}", bufs=8
                    )
                    nc.vector.tensor_tensor(
                        m_t[:].rearrange("p (z j) -> p z j", z=ZBLK),
                        ey.unsqueeze(1).broadcast_to((P_TILE, ZBLK, NXYZ)),
                        ez_b.unsqueeze(2).broadcast_to((P_TILE, ZBLK, NXYZ)),
                        AluOpType.mult,
                    )
                    nc.tensor.matmul(
                        accs[b][:],
                        exs[c][:],
                        m_t[:],
                        start=(c == 0),
                        stop=(c == n_chunks - 1),
                    )

                    if c == n_chunks - 1:
                        # drain piece b: psum -> sbuf on Scalar (idle after
                        # tables; keeps Vector on pure outer-product work)
                        dst = out_s[:, b * ZBLK * NXYZ : (b + 1) * ZBLK * NXYZ]
                        nc.scalar.copy(dst, accs[b][:])
                        if b % 2 == 1:
                            # one DMA per piece-pair, rotating queues (i-major;
                            # host transposes to (z,j,i) while summing partials)
                            lo = (b - 1) * ZBLK * NXYZ
                            hi = (b + 1) * ZBLK * NXYZ
                            pair = out_s[:, lo:hi]
                            dens_pc = dens_d.ap().rearrange(
                                "(i zj) -> i zj", i=NXYZ
                            )[:, lo:hi]
                            dma_eng = (nc.sync, nc.gpsimd, nc.scalar)[(b - 1) // 2]
                            with nc.allow_non_contiguous_dma("strided output store"):
                                dma_eng.dma_start(dens_pc, pair)

    nc.compile()
    return nc


def _host_prep(X, aw, bw, elements, C_expand, real_grid):
    ax = np.ascontiguousarray(real_grid[0:NXYZ, 0])
    ay = np.ascontiguousarray(real_grid[0 : NXYZ * NXYZ : NXYZ, 1])
    az = np.ascontiguousarray(real_grid[0 : G : NXYZ * NXYZ, 2])

    mask = (elements != 5) & (C_expand == 1)
    act = np.nonzero(mask)[0]
    # per-(atom, gaussian) flattened arrays over active atoms
    bw_p = bw[act].reshape(-1).astype(np.float64)
    aw_p = aw[act].reshape(-1).astype(np.float64)
    x_p = np.repeat(X[act, 0].astype(np.float64), 6)
    y_p = np.repeat(X[act, 1].astype(np.float64), 6)
    z_p = np.repeat(X[act, 2].astype(np.float64), 6)
    p_act = bw_p.shape[0]

    per_core = max(1, math.ceil(p_act / (N_CORES * P_TILE))) * P_TILE
    n_chunks = per_core // P_TILE

    s_p = np.sqrt(-bw_p)
    coef_full = np.zeros((N_CORES * per_core, NCOEF), dtype=np.float32)
    coef_full[:, 4] = -1e4  # padding rows: exp(-1e4) -> 0
    coef_full[:p_act, 0] = s_p
    coef_full[:p_act, 1] = -s_p * x_p
    coef_full[:p_act, 2] = -s_p * y_p
    coef_full[:p_act, 3] = -s_p * z_p
    coef_full[:p_act, 4] = np.log(aw_p)

    # core/chunk/partition layout: [core][chunk][row(128)] -> [row, chunk*NCOEF+j]
    coefs = []
    for core in range(N_CORES):
        cc = coef_full[core * per_core : (core + 1) * per_core]  # [per_core, NCOEF]
        cc = cc.reshape(n_chunks, P_TILE, NCOEF).transpose(1, 0, 2).reshape(P_TILE, -1)
        coefs.append(np.ascontiguousarray(cc))

    # packed per-core input: [axs (3*48) | coef (NCOEF*n_chunks)]
    inps = []
    for core in range(N_CORES):
        nco = NCOEF * n_chunks
        inp = np.empty((P_TILE, 3 * NXYZ + nco), dtype=np.float32)
        inp[:, 0:nco] = coefs[core]
        inp[:, nco : nco + NXYZ] = ay[None, :]
        inp[:, nco + NXYZ : nco + 2 * NXYZ] = az[None, :]
        inp[:, nco + 2 * NXYZ :] = ax[None, :]
        inps.append(inp)
    return inps, n_chunks


_prog_cache = {}


def kernel(X, aw, bw, elements, C_expand, real_grid, _trace=False):
    from concourse import bass_utils

    X = np.asarray(X)
    aw = np.asarray(aw)
    bw = np.asarray(bw)
    elements = np.asarray(elements)
    C_expand = np.asarray(C_expand)
    real_grid = np.asarray(real_grid)

    inps, n_chunks = _host_prep(X, aw, bw, elements, C_expand, real_grid)

    key = (n_chunks, MM_DTYPE)
    if key not in _prog_cache:
        _prog_cache[key] = _build_program(n_chunks)
    nc = _prog_cache[key]

    in_maps = [{"inp": inps[core]} for core in range(N_CORES)]
    res = bass_utils.run_bass_kernel_spmd(
        nc, in_maps, core_ids=list(range(N_CORES)), trace=_trace
    )
    dens = np.zeros((G2D, NXYZ), dtype=np.float64)
    for core in range(N_CORES):
        dens += res.results[core]["dens"].reshape(NXYZ, G2D).T
    out = np.ascontiguousarray(dens.reshape(-1)).astype(np.float32)
    if _trace:
        return out, res
    return out



# revision 2
# speedup vs baseline: 1.0396x; 1.0396x over previous
"""Gaussian density-grid kernel for Trainium2 (8 NeuronCores).

density[g] = sum_{a,n} aw[a,n]*mask[a] * exp(bw[a,n] * ||grid_g - X_a||^2)

The grid is a regular 48^3 lattice, so the Gaussian factorizes per axis:
    exp(bw*(dx^2+dy^2+dz^2)) = Ex(i) * Ey(j) * Ez(k)
The three 48-entry 1D tables per (atom, gaussian) pair p are precomputed on
the host (bf16, weight folded into Ex) and DMA'd in.  On device, per z-piece
we form the outer product Ey (x) Ez (Vector + Pool engines) and contract
over p with the tensor engine:  out[i, (k,j)] = sum_p Ex[p,i] * (Ey*Ez)[p,(k,j)].

Active (mask==1) atoms are compacted on the host and the p axis is sharded
across the 8 cores; each core produces a partial density over the full grid
and the host sums the 8 partials.
"""

import math

import numpy as np

NXYZ = 48
G2D = NXYZ * NXYZ  # 2304 (k,j) pairs
G = NXYZ * G2D
N_CORES = 8
P_TILE = 128
TBL = 3 * NXYZ  # ex | ey | ez per chunk
ZBLK = 8  # z-rows per piece -> N = ZBLK*48 = 384 per matmul
NPIECE = NXYZ // ZBLK  # 6 matmul pieces per chunk
N_WARM = 6  # PE clock-ramp matmuls while the input DMA is in flight


def _build_program(n_chunks: int):
    import concourse.mybir as mybir
    import concourse.tile as tile
    from concourse import bacc
    from concourse.alu_op_type import AluOpType

    f32 = mybir.dt.float32
    bf16 = mybir.dt.bfloat16

    nc = bacc.Bacc(
        "TRN2",
        target_bir_lowering=False,
        debug=False,
        enable_asserts=False,
        num_devices=N_CORES,
    )

    inp_d = nc.dram_tensor("inp", [P_TILE, TBL * n_chunks], bf16, kind="ExternalInput")
    dens_d = nc.dram_tensor("dens", [G], f32, kind="ExternalOutput")

    with tile.TileContext(nc) as tc:
        with (
            tc.tile_pool(name="const", bufs=1) as cpool,
            tc.tile_pool(name="mbuf", bufs=1) as mpool,
            tc.tile_pool(name="outs", bufs=1) as opool,
            tc.tile_pool(name="acc", bufs=1, space="PSUM") as acc_pool,
        ):
            # PE warm-up: dep-free matmuls on zeroed scratch while the input
            # DMA is in flight, so the p-state ramps (0.65 -> 1.2+ GHz)
            # before the real matmuls
            warm_l = cpool.tile([P_TILE, NXYZ], bf16, name="warm_l")
            warm_r = cpool.tile([P_TILE, ZBLK * NXYZ], bf16, name="warm_r")
            nc.vector.memset(warm_l[:].bitcast(f32), 0.0)
            nc.vector.memset(warm_r[:].bitcast(f32), 0.0)
            warm_ps = acc_pool.tile(
                [NXYZ, ZBLK * NXYZ], f32, tag="warmps", name="warmps"
            )
            for _ in range(N_WARM):
                nc.tensor.matmul(warm_ps[:], warm_l[:], warm_r[:], start=True, stop=True)

            # per-chunk host-built tables, one DMA per chunk on its own queue
            tbls = []
            for c in range(n_chunks):
                t = cpool.tile([P_TILE, TBL], bf16, name=f"tbl{c}")
                eng = (nc.sync, nc.scalar, nc.gpsimd)[c % 3]
                eng.dma_start(t[:], inp_d.ap()[:, c * TBL : (c + 1) * TBL])
                tbls.append(t)

            accs = [
                acc_pool.tile([NXYZ, ZBLK * NXYZ], f32, tag=f"acc{b}", name=f"acc{b}")
                for b in range(NPIECE)
            ]
            out_s = opool.tile([NXYZ, G2D], f32)

            # piece-outer loop: accumulate both chunks into PSUM, then drain
            # and DMA the piece immediately.  Pieces 1 and 4 (by emission
            # order) build their outer products on the otherwise-idle Pool
            # engine; the rest on Vector.
            order = (0, 4, 1, 2, 5, 3)
            pool_pieces = (4, 5)
            dens_v = dens_d.ap().rearrange("(i zj) -> i zj", i=NXYZ)
            dma_engs = (nc.sync, nc.gpsimd, nc.scalar, nc.sync, nc.gpsimd, nc.scalar)
            for bi, b in enumerate(order):
                for c in range(n_chunks):
                    ey = tbls[c][:, NXYZ : 2 * NXYZ]
                    ez_b = tbls[c][:, 2 * NXYZ + b * ZBLK : 2 * NXYZ + (b + 1) * ZBLK]
                    m_t = mpool.tile(
                        [P_TILE, ZBLK * NXYZ],
                        bf16,
                        tag="m",
                        name=f"m{b}_{c}",
                        bufs=NPIECE * n_chunks,
                    )
                    eng = nc.gpsimd if b in pool_pieces else nc.vector
                    eng.tensor_tensor(
                        m_t[:].rearrange("p (z j) -> p z j", z=ZBLK),
                        ey.unsqueeze(1).broadcast_to((P_TILE, ZBLK, NXYZ)),
                        ez_b.unsqueeze(2).broadcast_to((P_TILE, ZBLK, NXYZ)),
                        AluOpType.mult,
                    )
                    nc.tensor.matmul(
                        accs[b][:],
                        tbls[c][:, 0:NXYZ],
                        m_t[:],
                        start=(c == 0),
                        stop=(c == n_chunks - 1),
                    )
                # drain piece b: psum -> sbuf on Scalar (no activations in
                # this kernel otherwise), then straight to HBM
                dst = out_s[:, b * ZBLK * NXYZ : (b + 1) * ZBLK * NXYZ]
                nc.scalar.copy(dst, accs[b][:])
                with nc.allow_non_contiguous_dma("strided output store"):
                    dma_engs[bi].dma_start(
                        dens_v[:, b * ZBLK * NXYZ : (b + 1) * ZBLK * NXYZ], dst
                    )

    nc.compile()
    return nc


def _host_prep(X, aw, bw, elements, C_expand, real_grid):
    from ml_dtypes import bfloat16

    ax = real_grid[0:NXYZ, 0].astype(np.float64)
    ay = real_grid[0 : NXYZ * NXYZ : NXYZ, 1].astype(np.float64)
    az = real_grid[0 : G : NXYZ * NXYZ, 2].astype(np.float64)

    mask = (elements != 5) & (C_expand == 1)
    act = np.nonzero(mask)[0]
    # per-(atom, gaussian) flattened arrays over active atoms
    bw_p = bw[act].reshape(-1).astype(np.float64)
    aw_p = aw[act].reshape(-1).astype(np.float64)
    x_p = np.repeat(X[act, 0].astype(np.float64), 6)
    y_p = np.repeat(X[act, 1].astype(np.float64), 6)
    z_p = np.repeat(X[act, 2].astype(np.float64), 6)
    p_act = bw_p.shape[0]

    per_core = max(1, math.ceil(p_act / (N_CORES * P_TILE))) * P_TILE
    n_chunks = per_core // P_TILE
    n_pad = N_CORES * per_core

    def tables(coord, centers, weight=None):
        d = coord[None, :] - centers[:, None]
        t = np.exp(bw_p[:, None] * d * d)
        if weight is not None:
            t *= weight[:, None]
        full = np.zeros((n_pad, NXYZ), dtype=np.float64)
        full[:p_act] = t
        return full

    ex = tables(ax, x_p, aw_p)
    ey = tables(ay, y_p)
    ez = tables(az, z_p)
    tbl = np.concatenate([ex, ey, ez], axis=1)  # [n_pad, 144]

    # core/chunk/partition layout: pair (core, c, r) -> inp[r, c*TBL:(c+1)*TBL]
    inps = []
    for core in range(N_CORES):
        cc = tbl[core * per_core : (core + 1) * per_core]  # [per_core, TBL]
        cc = cc.reshape(n_chunks, P_TILE, TBL).transpose(1, 0, 2).reshape(P_TILE, -1)
        inps.append(np.ascontiguousarray(cc.astype(bfloat16)))
    return inps, n_chunks


_prog_cache = {}


def kernel(X, aw, bw, elements, C_expand, real_grid, _trace=False):
    from concourse import bass_utils

    X = np.asarray(X)
    aw = np.asarray(aw)
    bw = np.asarray(bw)
    elements = np.asarray(elements)
    C_expand = np.asarray(C_expand)
    real_grid = np.asarray(real_grid)

    inps, n_chunks = _host_prep(X, aw, bw, elements, C_expand, real_grid)

    if n_chunks not in _prog_cache:
        _prog_cache[n_chunks] = _build_program(n_chunks)
    nc = _prog_cache[n_chunks]

    in_maps = [{"inp": inps[core]} for core in range(N_CORES)]
    res = bass_utils.run_bass_kernel_spmd(
        nc, in_maps, core_ids=list(range(N_CORES)), trace=_trace
    )
    dens = np.zeros((G2D, NXYZ), dtype=np.float64)
    for core in range(N_CORES):
        dens += res.results[core]["dens"].reshape(NXYZ, G2D).T
    out = np.ascontiguousarray(dens.reshape(-1)).astype(np.float32)
    if _trace:
        return out, res
    return out


# revision 3
# speedup vs baseline: 1.1147x; 1.0722x over previous
"""Gaussian density-grid kernel for Trainium2 (8 NeuronCores).

density[g] = sum_{a,n} aw[a,n]*mask[a] * exp(bw[a,n] * ||grid_g - X_a||^2)

The grid is a regular 48^3 lattice, so the Gaussian factorizes per axis:
    exp(bw*(dx^2+dy^2+dz^2)) = Ex(i) * Ey(j) * Ez(k)
The three 1D tables per (atom, gaussian) pair p are precomputed on the host
(bf16, weight folded into Ex) and DMA'd in.  Active pairs are z-sorted and
sharded across the 8 cores, so each core's pairs cover only a narrow z
window (Gaussians decay fast); each core computes a compact [48, W*48]
output block and the host scatters it into the full grid while summing.

On device, per z-piece of 8 rows we form the outer product Ey (x) Ez
(Vector engine, last piece on the Pool engine) and contract over p with the
tensor engine:  out[i, (k,j)] = sum_p Ex[p,i] * (Ey*Ez)[p,(k,j)].
"""

import math

import numpy as np

NXYZ = 48
G2D = NXYZ * NXYZ  # 2304 (k,j) pairs
G = NXYZ * G2D
N_CORES = 8
P_TILE = 128
ZBLK = 8  # z-rows per piece -> N = ZBLK*48 = 384 per matmul
TAU = 3e-4  # truncation threshold for the per-pair z support
N_WARM = 4  # PE clock-ramp matmuls while the input DMA is in flight


def _build_program(n_chunks: int, npiece: int):
    import concourse.mybir as mybir
    import concourse.tile as tile
    from concourse import bacc
    from concourse.alu_op_type import AluOpType

    f32 = mybir.dt.float32
    bf16 = mybir.dt.bfloat16

    zw = npiece * ZBLK  # z window cells
    tblw = 2 * NXYZ + zw  # ex | ey | ez(window) per chunk
    gout = zw * NXYZ  # output columns

    nc = bacc.Bacc(
        "TRN2",
        target_bir_lowering=False,
        debug=False,
        enable_asserts=False,
        num_devices=N_CORES,
    )

    inp_d = nc.dram_tensor("inp", [P_TILE, tblw * n_chunks], bf16, kind="ExternalInput")
    dens_d = nc.dram_tensor("dens", [NXYZ * gout], f32, kind="ExternalOutput")

    with tile.TileContext(nc) as tc:
        with (
            tc.tile_pool(name="const", bufs=1) as cpool,
            tc.tile_pool(name="mbuf", bufs=1) as mpool,
            tc.tile_pool(name="outs", bufs=1) as opool,
            tc.tile_pool(name="acc", bufs=1, space="PSUM") as acc_pool,
        ):
            # Engine warm-up while the input DMA is in flight: dep-free work
            # on zeroed scratch keeps utilization high so the activity
            # monitor unthrottles the clocks before the real pipeline, and
            # the PE p-state ramps (0.65 -> 1.2+ GHz).
            warm_l = cpool.tile([P_TILE, NXYZ], bf16, name="warm_l")
            warm_r = cpool.tile([P_TILE, ZBLK * NXYZ], bf16, name="warm_r")
            nc.vector.memset(warm_l[:].bitcast(f32), 0.0)
            nc.vector.memset(warm_r[:].bitcast(f32), 0.0)
            warm_ps = acc_pool.tile(
                [NXYZ, ZBLK * NXYZ], f32, tag="warmps", name="warmps"
            )
            for _ in range(N_WARM):
                nc.tensor.matmul(warm_ps[:], warm_l[:], warm_r[:], start=True, stop=True)
            for w in range(2):  # Vector warm: ends before the input lands
                nc.vector.tensor_tensor(
                    warm_r[:, 0 : 2 * NXYZ].rearrange("p (z j) -> p z j", z=2),
                    warm_l[:].unsqueeze(1).broadcast_to((P_TILE, 2, NXYZ)),
                    warm_l[:, 0:2].unsqueeze(2).broadcast_to((P_TILE, 2, NXYZ)),
                    AluOpType.mult,
                )
            nc.gpsimd.tensor_tensor(  # Pool warm
                warm_r[:].rearrange("p (z j) -> p z j", z=ZBLK),
                warm_l[:].unsqueeze(1).broadcast_to((P_TILE, ZBLK, NXYZ)),
                warm_l[:, 0:ZBLK].unsqueeze(2).broadcast_to((P_TILE, ZBLK, NXYZ)),
                AluOpType.mult,
            )

            # per-chunk host-built tables, one DMA per chunk on its own queue
            tbls = []
            for c in range(n_chunks):
                t = cpool.tile([P_TILE, tblw], bf16, name=f"tbl{c}")
                eng = (nc.sync, nc.scalar, nc.gpsimd)[c % 3]
                eng.dma_start(t[:], inp_d.ap()[:, c * tblw : (c + 1) * tblw])
                tbls.append(t)

            accs = [
                acc_pool.tile([NXYZ, ZBLK * NXYZ], f32, tag=f"acc{b}", name=f"acc{b}")
                for b in range(npiece)
            ]
            out_s = opool.tile([NXYZ, gout], f32)

            # piece-outer loop: accumulate both chunks into PSUM, then drain
            # and DMA the piece immediately.  The last piece builds its outer
            # products on the Pool engine (slow but fully slack-covered);
            # the rest on Vector.
            dens_v = dens_d.ap().rearrange("(i zj) -> i zj", i=NXYZ)
            dma_engs = (nc.sync, nc.gpsimd, nc.scalar, nc.sync, nc.gpsimd, nc.scalar)
            for b in range(npiece):
                for c in range(n_chunks):
                    ey = tbls[c][:, NXYZ : 2 * NXYZ]
                    ez_b = tbls[c][:, 2 * NXYZ + b * ZBLK : 2 * NXYZ + (b + 1) * ZBLK]
                    m_t = mpool.tile(
                        [P_TILE, ZBLK * NXYZ],
                        bf16,
                        tag="m",
                        name=f"m{b}_{c}",
                        bufs=npiece * n_chunks,
                    )
                    eng = nc.gpsimd if b == npiece - 1 else nc.vector
                    eng.tensor_tensor(
                        m_t[:].rearrange("p (z j) -> p z j", z=ZBLK),
                        ey.unsqueeze(1).broadcast_to((P_TILE, ZBLK, NXYZ)),
                        ez_b.unsqueeze(2).broadcast_to((P_TILE, ZBLK, NXYZ)),
                        AluOpType.mult,
                    )
                    nc.tensor.matmul(
                        accs[b][:],
                        tbls[c][:, 0:NXYZ],
                        m_t[:],
                        start=(c == 0),
                        stop=(c == n_chunks - 1),
                    )
                # drain piece b: psum -> sbuf on Scalar, then straight to HBM
                dst = out_s[:, b * ZBLK * NXYZ : (b + 1) * ZBLK * NXYZ]
                nc.scalar.copy(dst, accs[b][:])
                with nc.allow_non_contiguous_dma("strided output store"):
                    dma_engs[b].dma_start(
                        dens_v[:, b * ZBLK * NXYZ : (b + 1) * ZBLK * NXYZ], dst
                    )

    nc.compile()
    return nc


def _host_prep(X, aw, bw, elements, C_expand, real_grid):
    from ml_dtypes import bfloat16

    ax = real_grid[0:NXYZ, 0].astype(np.float64)
    ay = real_grid[0 : NXYZ * NXYZ : NXYZ, 1].astype(np.float64)
    az = real_grid[0 : G : NXYZ * NXYZ, 2].astype(np.float64)

    mask = (elements != 5) & (C_expand == 1)
    act = np.nonzero(mask)[0]
    # per-(atom, gaussian) flattened arrays over active atoms, z-sorted
    bw_p = bw[act].reshape(-1).astype(np.float64)
    aw_p = aw[act].reshape(-1).astype(np.float64)
    x_p = np.repeat(X[act, 0].astype(np.float64), 6)
    y_p = np.repeat(X[act, 1].astype(np.float64), 6)
    z_p = np.repeat(X[act, 2].astype(np.float64), 6)
    order = np.argsort(z_p, kind="stable")
    bw_p, aw_p = bw_p[order], aw_p[order]
    x_p, y_p, z_p = x_p[order], y_p[order], z_p[order]
    p_act = bw_p.shape[0]

    per_core = max(1, math.ceil(p_act / (N_CORES * P_TILE))) * P_TILE
    n_chunks = per_core // P_TILE

    # per-core z support window, in grid cells, piece-quantized
    h_p = np.sqrt(np.log(1.0 / TAU) / np.abs(bw_p))
    spacing = float(az[1] - az[0])
    z0s, widths = [], []
    for core in range(N_CORES):
        lo, hi = core * per_core, min((core + 1) * per_core, p_act)
        if lo >= p_act:
            z0s.append(0)
            widths.append(ZBLK)
            continue
        zlo = np.clip((z_p[lo:hi] - h_p[lo:hi]).min(), az[0], az[-1])
        zhi = np.clip((z_p[lo:hi] + h_p[lo:hi]).max(), az[0], az[-1])
        clo = int(np.floor(zlo / spacing))
        chi = min(int(np.ceil(zhi / spacing)) + 1, NXYZ)
        z0s.append(clo)
        widths.append(chi - clo)
    npiece = max(1, math.ceil(max(widths) / ZBLK))
    zw = npiece * ZBLK
    z0s = [min(z0, NXYZ - zw) for z0 in z0s]

    def tables(coord, centers, weight=None):
        d = coord[None, :] - centers[:, None]
        t = np.exp(bw_p[:, None] * d * d)
        if weight is not None:
            t *= weight[:, None]
        return t

    ex = tables(ax, x_p, aw_p)
    ey = tables(ay, y_p)
    ez = tables(az, z_p)

    tblw = 2 * NXYZ + zw
    inps = []
    for core in range(N_CORES):
        z0 = z0s[core]
        tbl = np.zeros((n_chunks, P_TILE, tblw), dtype=np.float64)
        lo = core * per_core
        n_here = max(0, min(per_core, p_act - lo))
        if n_here:
            sl = slice(lo, lo + n_here)
            flat = tbl.reshape(per_core, tblw)
            flat[:n_here, 0:NXYZ] = ex[sl]
            flat[:n_here, NXYZ : 2 * NXYZ] = ey[sl]
            flat[:n_here, 2 * NXYZ :] = ez[sl, z0 : z0 + zw]
        # pair (core, c, r) -> inp[r, c*tblw:(c+1)*tblw]
        cc = tbl.transpose(1, 0, 2).reshape(P_TILE, -1)
        inps.append(np.ascontiguousarray(cc.astype(bfloat16)))
    return inps, n_chunks, npiece, z0s


_prog_cache = {}


def kernel(X, aw, bw, elements, C_expand, real_grid, _trace=False):
    from concourse import bass_utils

    X = np.asarray(X)
    aw = np.asarray(aw)
    bw = np.asarray(bw)
    elements = np.asarray(elements)
    C_expand = np.asarray(C_expand)
    real_grid = np.asarray(real_grid)

    inps, n_chunks, npiece, z0s = _host_prep(
        X, aw, bw, elements, C_expand, real_grid
    )

    key = (n_chunks, npiece)
    if key not in _prog_cache:
        _prog_cache[key] = _build_program(n_chunks, npiece)
    nc = _prog_cache[key]

    in_maps = [{"inp": inps[core]} for core in range(N_CORES)]
    res = bass_utils.run_bass_kernel_spmd(
        nc, in_maps, core_ids=list(range(N_CORES)), trace=_trace
    )
    zw = npiece * ZBLK
    dens = np.zeros((NXYZ, G2D), dtype=np.float64)  # [i, (z,j)]
    for core in range(N_CORES):
        blk = res.results[core]["dens"].reshape(NXYZ, zw * NXYZ)
        z0 = z0s[core]
        dens[:, z0 * NXYZ : z0 * NXYZ + zw * NXYZ] += blk
    out = np.ascontiguousarray(dens.T.reshape(-1)).astype(np.float32)
    if _trace:
        return out, res
    return out
